# revision 4
# baseline (speedup 1.0000x reference)
"""DisBlock (2 resblocks + BN + global-mean + reparam sampling) on 8 Trainium2 cores.

Strategy: data-parallel over the batch (32 images -> 4 per core), weights
replicated. Training-mode BatchNorm needs global batch statistics, so each
core computes per-channel (sum, sumsq) partials and a tiny AllReduce (one per
conv output: conv1, shortcut, conv2, conv3, conv4) produces the global mean /
var before the affine+relu is applied.

All convolutions run as implicit GEMMs on the PE array in float32r (fp32 bits,
single-pass reduced-precision matmul, ~2e-4 rel err):
  - 3x3 stride-1 convs read 16x16 activations from zero-padded 18x18 SBUF
    tiles; each of the 9 taps is an accumulating matmul over 4 input-channel
    chunks of 128.
  - the stride-2 convs (conv1 3x3 s2, shortcut 1x1 s2) read a polyphase
    decomposition of the padded input (4 phases of 17x17) prepared on host, so
    every matmul rhs is a unit-stride window.

conv1/shortcut raw outputs are evacuated to SBUF so their stat AllReduces
overlap the next conv's matmuls; conv2/3/4 outputs dwell in PSUM (8 banks)
until their AllReduce lands, then scale/bias+relu (+residual add) is applied
on the way out. SBUF is phase-scoped: the polyphase input pool is closed after
the stride-2 convs so the padded-activation pool can reuse its space.
"""

import numpy as np

import concourse.bass as bass  # noqa: F401
import concourse.tile as tile
from concourse import bacc, mybir
from concourse.bass_utils import run_bass_kernel_spmd

N_CORES = 8
BPC = 4          # batch per core
CB = 4           # channel chunks of 128 (C = 512)
P = 128
S = 100          # samples
EPS = 1e-5
NHW = 32 * 256   # BN reduction size (global batch x 16 x 16)

F32 = mybir.dt.float32
FR = mybir.dt.float32r
AF = mybir.ActivationFunctionType
ALU = mybir.AluOpType
AX = mybir.AxisListType

_CACHE = {}

# gb vector order: [g1a, b1a, gs1, bs1, g1b, b1b, g2a, b2a, g2b, b2b]
GI = {0: 0, 1: 2, 2: 4, 3: 6, 4: 8}  # bn id -> gamma index (beta = +1)
# bn ids: 0=bn1a(conv1) 1=bn_sc(shortcut) 2=bn1b(conv2) 3=bn2a(conv3) 4=bn2b(conv4)


def _build():
    nc = bacc.Bacc("TRN2", target_bir_lowering=False, debug=False,
                   num_devices=N_CORES)

    xp_d = nc.dram_tensor("xp", [CB, P, 2, 2, BPC, 17, 17], F32, kind="ExternalInput")
    nz_d = nc.dram_tensor("nz", [CB, P, BPC * S], F32, kind="ExternalInput")
    gb_d = nc.dram_tensor("gb", [10, CB, P], F32, kind="ExternalInput")
    w_d = {
        "w1a": nc.dram_tensor("w1a", [CB, P, 4608], F32, kind="ExternalInput"),
        "w1b": nc.dram_tensor("w1b", [CB, P, 4608], F32, kind="ExternalInput"),
        "w2a": nc.dram_tensor("w2a", [CB, P, 4608], F32, kind="ExternalInput"),
        "w2b": nc.dram_tensor("w2b", [CB, P, 4608], F32, kind="ExternalInput"),
        "ws1": nc.dram_tensor("ws1", [CB, P, 512], F32, kind="ExternalInput"),
    }
    avg_d = nc.dram_tensor("avg", [CB, P, BPC], F32, kind="ExternalOutput")
    dis_d = nc.dram_tensor("dis", [CB, P, BPC], F32, kind="ExternalOutput")
    smp_d = nc.dram_tensor("smp", [CB, P, BPC * S], F32, kind="ExternalOutput")

    with tile.TileContext(nc) as tc:
        with (
            tc.tile_pool(name="pps", bufs=8, space="PSUM") as pps,
            tc.tile_pool(name="pw", bufs=3) as pw,
            tc.tile_pool(name="praw", bufs=CB) as praw,
            tc.tile_pool(name="p1", bufs=1) as p1,
            tc.tile_pool(name="p4", bufs=4) as p4,
            tc.tile_pool(name="pd", bufs=1, space="DRAM") as pd,
        ):
            gb_sb = p1.tile([P, 10, CB], F32, tag="gb")
            nc.sync.dma_start(gb_sb[:], gb_d[:].rearrange("v c p -> p v c"))

            # stats tiles + AR plumbing per bn
            st = [p1.tile([P, 2, CB, 2], F32, tag=f"st{b}", name=f"st{b}")
                  for b in range(5)]
            sc_, bi_ = {}, {}

            def all_reduce(bn):
                ari = pd.tile([P, 16], F32, tag=f"ari{bn}", name=f"ari{bn}")
                nc.sync.dma_start(ari[:], st[bn][:])
                aro = pd.tile([P, 16], F32, tag=f"aro{bn}", name=f"aro{bn}",
                              addr_space="Shared")
                nc.gpsimd.collective_compute(
                    "AllReduce", ALU.add,
                    replica_groups=[list(range(N_CORES))],
                    ins=[ari[:].opt()], outs=[aro[:].opt()])
                stg = p1.tile([P, 2, CB, 2], F32, tag=f"stg{bn}", name=f"stg{bn}")
                nc.sync.dma_start(stg[:], aro[:])
                return stg

            def bn_post(bn, stg):
                """Global (sum, sumsq) -> per-channel scale/bias tiles [P, CB]."""
                def t(nm):
                    return p1.tile([P, CB], F32, tag=f"{nm}{bn}", name=f"{nm}{bn}")
                sums = p1.tile([P, 2, CB], F32, tag=f"sums{bn}", name=f"sums{bn}")
                nc.vector.tensor_reduce(sums[:], stg[:], AX.X, ALU.add)
                mean, ex2, msq, var = t("mean"), t("ex2"), t("msq"), t("var")
                rec, inv, s, ms, b_ = t("rec"), t("inv"), t("s"), t("ms"), t("b")
                nc.scalar.mul(mean[:], sums[:, 0, :], 1.0 / NHW)
                nc.scalar.mul(ex2[:], sums[:, 1, :], 1.0 / NHW)
                nc.vector.tensor_mul(msq[:], mean[:], mean[:])
                nc.vector.tensor_sub(var[:], ex2[:], msq[:])
                nc.vector.tensor_scalar_add(var[:], var[:], EPS)
                nc.vector.reciprocal(rec[:], var[:])
                nc.scalar.sqrt(inv[:], rec[:])
                nc.vector.tensor_mul(s[:], inv[:], gb_sb[:, GI[bn], :])
                nc.vector.tensor_mul(ms[:], mean[:], s[:])
                nc.vector.tensor_sub(b_[:], gb_sb[:, GI[bn] + 1, :], ms[:])
                sc_[bn], bi_[bn] = s, b_

            def stats(bn, ps, ocb, nb, copy_to=None):
                """sum/sumsq of a PSUM group into st[bn] slices (ACT engine)."""
                if copy_to is None:
                    tr = p4.tile([P, 512], F32, tag="tmp", name="tr")
                    nc.scalar.activation(tr[:], ps[:], AF.Copy,
                                         accum_out=st[bn][:, 0, ocb, nb:nb + 1])
                else:
                    nc.scalar.activation(copy_to, ps[:], AF.Copy,
                                         accum_out=st[bn][:, 0, ocb, nb:nb + 1])
                tr2 = p4.tile([P, 512], F32, tag="tmp", name="tr2")
                nc.scalar.activation(tr2[:], ps[:], AF.Square,
                                     accum_out=st[bn][:, 1, ocb, nb:nb + 1])

            avg_sb = []
            c1r, scr = [], []
            with (
                tc.tile_pool(name="px", bufs=CB) as px,
                tc.tile_pool(name="pwsc", bufs=2) as pwsc,
            ):
                # ---------------- phase 0: x load + avg ----------------
                x_sb = []
                for cb in range(CB):
                    xt = px.tile([P, 2, 2, BPC, 17, 17], FR, tag="x", name=f"x{cb}")
                    nc.sync.dma_start(xt[:], xp_d[cb].bitcast(FR))
                    x_sb.append(xt)
                for cb in range(CB):
                    ar = p4.tile([P, BPC], F32, tag="ared", name=f"ared{cb}")
                    nc.vector.tensor_reduce(
                        ar[:],
                        x_sb[cb][:].bitcast(F32).rearrange(
                            "p di dj b i j -> p b di dj (i j)"),
                        AX.XYZ, ALU.add)
                    av = p4.tile([P, BPC], F32, tag="avg", name=f"avg{cb}")
                    nc.scalar.mul(av[:], ar[:], 1.0 / 1024.0)
                    avg_sb.append(av)
                    nc.sync.dma_start(avg_d[cb], av[:])

                # ---------------- phase 1: conv1 (3x3 s2, polyphase) --------
                for ocb in range(CB):
                    wt = pw.tile([P, 9, CB, P], FR, tag="w", name=f"w1a_{ocb}")
                    nc.sync.dma_start(
                        wt[:], w_d["w1a"][ocb].bitcast(FR).rearrange(
                            "p (o i f) -> p o i f", o=9, i=CB))
                    cr = praw.tile([P, 1024], F32, tag="c1r", name=f"c1r{ocb}")
                    c1r.append(cr)
                    for nb in range(2):
                        ps = pps.tile([P, 512], F32, tag="ps", name=f"ps1_{ocb}{nb}")
                        k = 0
                        for kh in range(3):
                            for kw in range(3):
                                for icb in range(CB):
                                    rhs = x_sb[icb][
                                        :, kh % 2, kw % 2, nb * 2:(nb + 1) * 2,
                                        kh // 2:kh // 2 + 16, kw // 2:kw // 2 + 16]
                                    nc.tensor.matmul(
                                        ps[:], wt[:, kh * 3 + kw, icb, :], rhs,
                                        start=(k == 0), stop=(k == 35))
                                    k += 1
                        stats(0, ps, ocb, nb,
                              copy_to=cr[:, nb * 512:(nb + 1) * 512])
                bn_post(0, all_reduce(0))

                # ---------------- phase 2: shortcut (1x1 s2) ----------------
                for ocb in range(CB):
                    ws = pwsc.tile([P, CB, P], FR, tag="wsc", name=f"ws1_{ocb}")
                    nc.sync.dma_start(
                        ws[:], w_d["ws1"][ocb].bitcast(FR).rearrange(
                            "p (i f) -> p i f", i=CB))
                    sr = praw.tile([P, 1024], F32, tag="scr", name=f"scr{ocb}")
                    scr.append(sr)
                    for nb in range(2):
                        ps = pps.tile([P, 512], F32, tag="ps", name=f"pss_{ocb}{nb}")
                        for icb in range(CB):
                            rhs = x_sb[icb][:, 1, 1, nb * 2:(nb + 1) * 2,
                                            0:16, 0:16]
                            nc.tensor.matmul(ps[:], ws[:, icb, :], rhs,
                                             start=(icb == 0), stop=(icb == 3))
                        stats(1, ps, ocb, nb,
                              copy_to=sr[:, nb * 512:(nb + 1) * 512])
                bn_post(1, all_reduce(1))
            # px/pwsc closed -> their SBUF is reusable by pa below

            with tc.tile_pool(name="pa", bufs=CB) as pa:
                # ------------- phase 1b: apply bn1a+relu -> a1 (padded) -----
                a1 = []
                for ocb in range(CB):
                    at = pa.tile([P, BPC, 18, 18], FR, tag="apad", name=f"a1_{ocb}")
                    nc.vector.memset(at[:].bitcast(F32), 0.0)
                    a1.append(at)
                for ocb in range(CB):
                    nc.scalar.activation(
                        a1[ocb][:, :, 1:17, 1:17],
                        c1r[ocb][:].rearrange("p (b h w) -> p b h w", b=BPC, h=16),
                        AF.Relu,
                        bias=bi_[0][:, ocb:ocb + 1], scale=sc_[0][:, ocb:ocb + 1])

                # ------------- phase 3: conv2 (3x3 s1), dwell in PSUM -------
                def conv3x3(wkey, src, bn, name):
                    groups = {}
                    for ocb in range(CB):
                        wt = pw.tile([P, 9, CB, P], FR, tag="w",
                                     name=f"{name}_{ocb}")
                        nc.sync.dma_start(
                            wt[:], w_d[wkey][ocb].bitcast(FR).rearrange(
                                "p (o i f) -> p o i f", o=9, i=CB))
                        for nb in range(2):
                            ps = pps.tile([P, 512], F32, tag="ps",
                                          name=f"ps_{name}{ocb}{nb}")
                            k = 0
                            for kh in range(3):
                                for kw in range(3):
                                    for icb in range(CB):
                                        rhs = src[icb][:, nb * 2:(nb + 1) * 2,
                                                       kh:kh + 16, kw:kw + 16]
                                        nc.tensor.matmul(
                                            ps[:], wt[:, kh * 3 + kw, icb, :],
                                            rhs, start=(k == 0), stop=(k == 35))
                                        k += 1
                            stats(bn, ps, ocb, nb)
                            groups[(ocb, nb)] = ps
                    return groups

                c2ps = conv3x3("w1b", a1, 2, "c2")
                bn_post(2, all_reduce(2))
                bcomb = p1.tile([P, CB], F32, tag="bcomb", name="bcomb")
                nc.vector.tensor_add(bcomb[:], bi_[2][:], bi_[1][:])

                # ------------- phase 3b: h1 = relu(bn1b(conv2) + bnsc(sc)) --
                h1 = []
                for ocb in range(CB):
                    ht = pa.tile([P, BPC, 18, 18], FR, tag="h1", name=f"h1_{ocb}")
                    nc.vector.memset(ht[:].bitcast(F32), 0.0)
                    h1.append(ht)
                for ocb in range(CB):
                    for nb in range(2):
                        t2 = p4.tile([P, 512], F32, tag="tmp", name="t2")
                        nc.scalar.activation(
                            t2[:], scr[ocb][:, nb * 512:(nb + 1) * 512],
                            AF.Identity,
                            scale=sc_[1][:, ocb:ocb + 1],
                            bias=bcomb[:, ocb:ocb + 1])
                        t3 = p4.tile([P, 512], F32, tag="tmp", name="t3")
                        nc.vector.scalar_tensor_tensor(
                            t3[:], c2ps[(ocb, nb)][:], sc_[2][:, ocb:ocb + 1],
                            t2[:], ALU.mult, ALU.add)
                        nc.scalar.activation(
                            h1[ocb][:, nb * 2:(nb + 1) * 2, 1:17, 1:17],
                            t3[:].rearrange("p (b h w) -> p b h w", b=2, h=16),
                            AF.Relu)

                # ------------- phase 4: conv3 + apply -> a3 -----------------
                c3ps = conv3x3("w2a", h1, 3, "c3")
                bn_post(3, all_reduce(3))
                a3 = []
                for ocb in range(CB):
                    at = pa.tile([P, BPC, 18, 18], FR, tag="apad",
                                 name=f"a3_{ocb}")
                    nc.vector.memset(at[:].bitcast(F32), 0.0)
                    a3.append(at)
                for ocb in range(CB):
                    for nb in range(2):
                        nc.scalar.activation(
                            a3[ocb][:, nb * 2:(nb + 1) * 2, 1:17, 1:17],
                            c3ps[(ocb, nb)][:].rearrange(
                                "p (b h w) -> p b h w", b=2, h=16),
                            AF.Relu,
                            bias=bi_[3][:, ocb:ocb + 1],
                            scale=sc_[3][:, ocb:ocb + 1])

                # ------------- phase 5: conv4 + final -----------------------
                c4ps = conv3x3("w2b", a3, 4, "c4")
                bn_post(4, all_reduce(4))

                for ocb in range(CB):
                    nt = p4.tile([P, BPC * S], F32, tag="nz", bufs=2,
                                 name=f"nz{ocb}")
                    nc.sync.dma_start(nt[:], nz_d[ocb])
                    dr = p4.tile([P, BPC], F32, tag="disr", name=f"disr{ocb}")
                    for nb in range(2):
                        t3 = p4.tile([P, 512], F32, tag="tmp", name="f3")
                        for bi in range(2):
                            nc.vector.scalar_tensor_tensor(
                                t3[:, bi * 256:(bi + 1) * 256].rearrange(
                                    "p (h w) -> p h w", h=16),
                                c4ps[(ocb, nb)][:, bi * 256:(bi + 1) * 256]
                                .rearrange("p (h w) -> p h w", h=16),
                                sc_[4][:, ocb:ocb + 1],
                                h1[ocb][:, nb * 2 + bi, 1:17, 1:17].bitcast(F32),
                                ALU.mult, ALU.add)
                        t4 = p4.tile([P, 512], F32, tag="tmp", name="f4")
                        nc.scalar.activation(t4[:], t3[:], AF.Relu,
                                             bias=bi_[4][:, ocb:ocb + 1])
                        nc.vector.tensor_reduce(
                            dr[:, nb * 2:(nb + 1) * 2],
                            t4[:].rearrange("p (b k) -> p b k", b=2),
                            AX.X, ALU.add)
                    ds = p4.tile([P, BPC], F32, tag="dis", name=f"dis{ocb}")
                    nc.scalar.mul(ds[:], dr[:], 1.0 / 256.0)
                    nc.sync.dma_start(dis_d[ocb], ds[:])
                    sm = p4.tile([P, BPC * S], F32, tag="smp", bufs=2,
                                 name=f"smp{ocb}")
                    for b in range(BPC):
                        nc.vector.tensor_scalar(
                            sm[:, b * S:(b + 1) * S],
                            nt[:, b * S:(b + 1) * S],
                            ds[:, b:b + 1], avg_sb[ocb][:, b:b + 1],
                            ALU.mult, ALU.add)
                    nc.sync.dma_start(smp_d[ocb], sm[:])

    nc.compile()
    return nc


def _prep_w3(w):
    # [oc, ic, 3, 3] -> [CB(ocb), P(icp), (off, icb, ocf)]
    w6 = np.asarray(w, np.float32).reshape(CB, P, CB, P, 3, 3)
    wt = w6.transpose(0, 3, 4, 5, 2, 1)          # [ocb, icp, kh, kw, icb, ocf]
    return np.ascontiguousarray(wt).reshape(CB, P, 4608)


def _prep_w1(w):
    w4 = np.asarray(w, np.float32).reshape(CB, P, CB, P)
    wt = w4.transpose(0, 3, 2, 1)                # [ocb, icp, icb, ocf]
    return np.ascontiguousarray(wt).reshape(CB, P, 512)


def kernel(x, noise, w1a, g1a, b1a, w1b, g1b, b1b, ws1, gs1, bs1,
           w2a, g2a, b2a, w2b, g2b, b2b):
    x = np.asarray(x, np.float32)
    noise = np.asarray(noise, np.float32)

    if "nc" not in _CACHE:
        _CACHE["nc"] = _build()
    nc = _CACHE["nc"]

    wmaps = {
        "w1a": _prep_w3(w1a), "w1b": _prep_w3(w1b),
        "w2a": _prep_w3(w2a), "w2b": _prep_w3(w2b),
        "ws1": _prep_w1(ws1),
    }
    gb = np.stack([np.asarray(v, np.float32) for v in
                   (g1a, b1a, gs1, bs1, g1b, b1b, g2a, b2a, g2b, b2b)])
    gb = np.ascontiguousarray(gb).reshape(10, CB, P)

    in_maps = []
    for i in range(N_CORES):
        b0 = i * BPC
        xs = x[b0:b0 + BPC]                                  # [4, 512, 32, 32]
        xpad = np.pad(xs, ((0, 0), (0, 0), (1, 1), (1, 1)))  # [4, 512, 34, 34]
        ph = xpad.reshape(BPC, 512, 17, 2, 17, 2)            # [b, c, i, di, j, dj]
        xt = np.ascontiguousarray(ph.transpose(1, 3, 5, 0, 2, 4))
        xp_i = xt.reshape(CB, P, 2, 2, BPC, 17, 17)
        nz_i = np.ascontiguousarray(
            noise[:, b0:b0 + BPC, :].transpose(2, 1, 0)).reshape(CB, P, BPC * S)
        in_maps.append({"xp": xp_i, "nz": nz_i, "gb": gb, **wmaps})

    res = run_bass_kernel_spmd(nc, in_maps, list(range(N_CORES)))
    _CACHE["last_results"] = res

    avg = np.empty((32, 512), np.float32)
    dis = np.empty((32, 512), np.float32)
    samples = np.empty((S, 32, 512), np.float32)
    for i in range(N_CORES):
        b0 = i * BPC
        r = res.results[i]
        avg[b0:b0 + BPC] = r["avg"].transpose(2, 0, 1).reshape(BPC, 512)
        dis[b0:b0 + BPC] = r["dis"].transpose(2, 0, 1).reshape(BPC, 512)
        smp = r["smp"].reshape(CB, P, BPC, S)                # [cb, p, b, s]
        samples[:, b0:b0 + BPC, :] = smp.transpose(3, 2, 0, 1).reshape(S, BPC, 512)
    return (avg, dis, samples)
